# revision 1
# baseline (speedup 1.0000x reference)
"""Trainium2 Bass kernel for nn_AttentionModule_53223234732422.

Computes: RMSNorm -> QKV projections -> interleaved-pair RoPE on Q,K ->
causal softmax attention (16 heads, head_dim 128) -> output projection.

Sharding (8 NeuronCores, tensor parallel over heads):
  - every core computes the RMSNorm (cheap, avoids an activation collective),
  - each core owns 2 heads: QKV projections with column-sliced weights,
    RoPE, causal attention for those heads,
  - per-head context is AllGathered (2 x 1 MiB fp32 per rank),
  - output projection is split column-wise: each core produces 256 output
    features from the full gathered context.

Host-side preparation (layout only):
  - xs transposed to feature-major [E, S] so contractions land on SBUF
    partitions,
  - norm_w folded into the QKV weights,
  - wq/wk rows permuted per head so RoPE pairs are deinterleaved
    (x0 rows 0..63, x1 rows 64..127); scores are permutation invariant,
  - weights pre-rounded to fp32r (11 mantissa bits, RNE) to match the
    on-device rounding path,
  - cos/sin tables (fp16-arange thetas, like the reference) and the 4
    diagonal causal-mask tiles precomputed.

Dtypes: all matmuls run fp32r (full-rate fp32 path on the PE, 11 mantissa
bits, ~1.2e-4 input rounding; plain fp32 would be 4x slower). PSUM
accumulation is fp32 everywhere. Every tensor an fp32r matmul consumes is
written only by fp32r-typed producers (BIR verifier requirement); weights
are pre-rounded on the host and DMA'd with fp32r-typed endpoints.
Measured on HW: end-to-end relative error 2.35e-4 vs the fp32 reference.
"""

import sys

sys.path.insert(0, "/opt/trn_rl_repo")

import numpy as np

import concourse.bacc as bacc
import concourse.mybir as mybir
import concourse.tile as tile
from concourse.bass import ds, ts

dt = mybir.dt
AF = mybir.ActivationFunctionType
ALU = mybir.AluOpType

S = 2048
E = 2048
H = 16
D = 128
HALF = D // 2
EPS = 1e-6
THETA = 10000.0
N_CORES = 8
HPC = H // N_CORES  # heads per core
JC = HPC * D  # 256: local q/k/v width
EB = E // N_CORES  # 256: output columns per core
ET = E // 128  # 16 feature tiles
TT = S // 128  # 16 token tiles
NS = S // 512  # 4 token strips
CH = ET // 2  # 8 e-tiles per contraction chunk
INV_SQRT_D = float(1.0 / np.sqrt(np.float32(D)))

F32 = dt.float32
F32R = dt.float32r
BF16 = dt.bfloat16

_NC_CACHE = {}


def _build_nc():
    nc = bacc.Bacc(trn_type="TRN2", num_devices=N_CORES)

    xsT = nc.dram_tensor("xsT", [E, S], F32R, kind="ExternalInput")
    wqT = nc.dram_tensor("wqT", [E, JC], F32R, kind="ExternalInput")
    wkT = nc.dram_tensor("wkT", [E, JC], F32R, kind="ExternalInput")
    wvT = nc.dram_tensor("wvT", [E, JC], F32R, kind="ExternalInput")
    woT = nc.dram_tensor("woT", [E, EB], F32R, kind="ExternalInput")
    cosF = nc.dram_tensor("cosF", [D, S], F32, kind="ExternalInput")
    sinF = nc.dram_tensor("sinF", [D, S], F32, kind="ExternalInput")
    masksT = nc.dram_tensor("masks", [NS, 128, 512], F32, kind="ExternalInput")
    out_ext = nc.dram_tensor("out", [S, EB], F32, kind="ExternalOutput")

    rg = [list(range(N_CORES))]

    with tile.TileContext(nc) as tc:
        with (
            tc.tile_pool(name="persist", bufs=1) as pp,
            tc.tile_pool(name="dram", bufs=1, space="DRAM") as dpool,
        ):
            ones_f = pp.tile([128, 1], F32, tag="ones_f")
            ones_r = pp.tile([128, 1], F32R, tag="ones_r")
            ones_b = pp.tile([128, 1], BF16, tag="ones_b")
            eps_sc = pp.tile([1, 1], F32, tag="eps_sc")
            nc.vector.memset(ones_f[:], 1.0)
            nc.vector.tensor_copy(ones_r[:], ones_f[:])
            nc.vector.tensor_copy(ones_b[:], ones_f[:])
            nc.vector.memset(eps_sc[:], EPS)

            # RoPE'd q/k (fp32r; written only by the final rope add) and
            # bf16 token-major v.
            qrope = pp.tile([128, HPC * S], F32R, tag="qrope")
            krope = pp.tile([128, HPC * S], F32R, tag="krope")
            v_sb = pp.tile([128, TT * JC], F32R, tag="v_sb")

            cbounce = [
                dpool.tile([128, S], F32R, tag=f"cb{m}", name=f"cb{m}")
                for m in range(HPC)
            ]
            ag_out = [
                dpool.tile(
                    [N_CORES * 128, S],
                    F32R,
                    addr_space="Shared",
                    tag=f"ag{m}",
                    name=f"ag{m}",
                )
                for m in range(HPC)
            ]

            # ---------------- Phase A: sum-of-squares + rms ----------------
            # The 1/rms per-token scale commutes out of the e-contraction, so
            # QKV consumes RAW xs (host-pre-rounded to fp32r); the scale is
            # folded into the cos/sin tables (q/k) and the v evictions.
            with tc.tile_pool(name="bcC", bufs=1) as bcp:
                bcastR = bcp.tile([128, S], F32, tag="bcastR")
                recip_col = bcp.tile([128, TT], F32, tag="recip_col")
                with tc.tile_pool(name="xsp", bufs=CH + 1) as xsp:
                    kept = {}  # e -> resident xs tile (chunk-0 reuse)
                    with tc.tile_pool(name="rmsp", bufs=1) as rmsp:
                        rms_row = rmsp.tile([1, S], F32, tag="rms_row")
                        with (
                            tc.tile_pool(name="sqp", bufs=2) as sqp,
                            tc.tile_pool(name="psA", bufs=NS, space="PSUM") as psA,
                        ):
                            ssq_ps = [
                                psA.tile([1, 512], F32, tag="ssq", name="ssq")
                                for _ in range(NS)
                            ]
                            order = list(range(CH, ET)) + list(range(CH))
                            for idx, e in enumerate(order):
                                xt = xsp.tile([128, S], F32R, tag="xsA", name="xsA")
                                nc.sync.dma_start(xt[:], xsT[ts(e, 128), :])
                                if e < CH:
                                    kept[e] = xt
                                sq = sqp.tile([128, S], F32R, tag="sq")
                                nc.vector.tensor_mul(
                                    sq[:], xt[:].bitcast(F32), xt[:].bitcast(F32)
                                )
                                for s in range(NS):
                                    nc.tensor.matmul(
                                        ssq_ps[s][:],
                                        ones_r[:],
                                        sq[:, ts(s, 512)],
                                        start=(idx == 0),
                                        stop=(idx == ET - 1),
                                    )
                            for s in range(NS):
                                # rms = sqrt(ssq/E + eps)
                                nc.scalar.activation(
                                    rms_row[0:1, ts(s, 512)],
                                    ssq_ps[s][:],
                                    AF.Sqrt,
                                    bias=eps_sc[0:1, 0:1],
                                    scale=1.0 / E,
                                )
                        nc.vector.reciprocal(rms_row[:], rms_row[:])
                        nc.gpsimd.partition_broadcast(bcastR[:], rms_row[0:1, :])
                        # token-major view of the recips for the v scaling;
                        # bounce via DRAM so the strided gather runs on the
                        # DRAM side of the DMA.
                        rrow_d = dpool.tile([1, S], F32, tag="rrow_d", name="rrow_d")
                        nc.sync.dma_start(rrow_d[:], rms_row[:])
                        nc.sync.dma_start(
                            recip_col[:],
                            rrow_d[0, :].rearrange("(a p) -> p a", p=128),
                        )

                    # ------------ Phase C: QKV (2-chunk contraction) -------
                    with (
                        tc.tile_pool(name="wch", bufs=3) as wchp,
                        tc.tile_pool(name="acc", bufs=1) as accp,
                        tc.tile_pool(name="trig", bufs=1) as trigp,
                        tc.tile_pool(name="rsw", bufs=2) as rsp,
                        tc.tile_pool(name="psQK", bufs=4, space="PSUM") as psQK,
                        tc.tile_pool(name="psV", bufs=3, space="PSUM") as psV,
                    ):
                        cos_sb = trigp.tile([D, S], F32, tag="cos_sb")
                        sin_sb = trigp.tile([D, S], F32, tag="sin_sb")
                        nc.sync.dma_start(cos_sb[:], cosF[:])
                        nc.sync.dma_start(sin_sb[:], sinF[:])
                        # fold 1/rms into the rope tables
                        nc.vector.tensor_mul(cos_sb[:], cos_sb[:], bcastR[:])
                        nc.vector.tensor_mul(sin_sb[:], sin_sb[:], bcastR[:])

                        qacc = accp.tile([128, HPC * S], F32, tag="qacc")
                        kacc = accp.tile([128, HPC * S], F32, tag="kacc")

                        for chunk in range(2):
                            # weights for this chunk: [128, CH*JC], e-tile i
                            # at cols i*JC.
                            wtiles = []
                            for wdram in (wqT, wkT, wvT):
                                wc = wchp.tile(
                                    [128, CH * JC], F32R, tag="wch",
                                    name=f"w{chunk}_{wdram.name}",
                                )
                                src = wdram[ds(chunk * CH * 128, CH * 128), :]
                                nc.sync.dma_start(
                                    wc[:].rearrange("p (a j) -> p a j", a=CH),
                                    src.rearrange("(a p) j -> p a j", p=128),
                                )
                                wtiles.append(wc)
                            wq_c, wk_c, wv_c = wtiles

                            if chunk == 0:
                                xh = [kept[i] for i in range(CH)]
                            else:
                                xh = []
                                for i in range(CH):
                                    e = CH + i
                                    xt = xsp.tile(
                                        [128, S], F32R, tag="xsA", name="xsA"
                                    )
                                    nc.sync.dma_start(xt[:], xsT[ts(e, 128), :])
                                    xh.append(xt)

                            # q and k projections -> d-major [j, t]
                            for wc, acc, rope_dst in (
                                (wq_c, qacc, qrope),
                                (wk_c, kacc, krope),
                            ):
                                for m in range(HPC):
                                    for s in range(NS):
                                        ps = psQK.tile(
                                            [128, 512], F32, tag="qk_ps", name="qk_ps"
                                        )
                                        for i in range(CH):
                                            nc.tensor.matmul(
                                                ps[:],
                                                wc[:, ds(i * JC + m * D, D)],
                                                xh[i][:, ts(s, 512)],
                                                start=(i == 0),
                                                stop=(i == CH - 1),
                                            )
                                        asl = acc[:, ds(m * S + s * 512, 512)]
                                        if chunk == 0:
                                            nc.vector.tensor_copy(asl, ps[:])
                                        else:
                                            nc.vector.scalar_tensor_tensor(
                                                asl, ps[:], 1.0, asl, ALU.mult, ALU.add
                                            )
                                            # RoPE: r = cos*q + sin*swap64(q)
                                            sw = rsp.tile(
                                                [128, 512], F32, tag="rsw", name="rsw"
                                            )
                                            nc.vector.tensor_copy(
                                                sw[0:64, :], asl[64:128, :]
                                            )
                                            nc.vector.tensor_copy(
                                                sw[64:128, :], asl[0:64, :]
                                            )
                                            nc.vector.tensor_mul(
                                                asl, asl, cos_sb[:, ts(s, 512)]
                                            )
                                            nc.vector.tensor_mul(
                                                sw[:], sw[:], sin_sb[:, ts(s, 512)]
                                            )
                                            nc.vector.tensor_tensor(
                                                rope_dst[:, ds(m * S + s * 512, 512)],
                                                asl,
                                                sw[:],
                                                ALU.add,
                                            )

                            # v projection -> token-major [t, j], scaled by
                            # 1/rms[t] (per-partition scalar) at eviction
                            for t in range(TT):
                                ps = psV.tile([128, JC], F32, tag="v_ps", name="v_ps")
                                for i in range(CH):
                                    nc.tensor.matmul(
                                        ps[:],
                                        xh[i][:, ts(t, 128)],
                                        wv_c[:, ts(i, JC)],
                                        start=(i == 0),
                                        stop=(i == CH - 1),
                                    )
                                vsl = v_sb[:, ts(t, JC)]
                                rc = recip_col[:, t : t + 1]
                                if chunk == 0:
                                    nc.vector.tensor_scalar_mul(vsl, ps[:], rc)
                                else:
                                    nc.vector.scalar_tensor_tensor(
                                        vsl,
                                        ps[:],
                                        rc,
                                        vsl.bitcast(F32),
                                        ALU.mult,
                                        ALU.add,
                                    )

            # ---------------- Phase D: attention ----------------
            with (
                tc.tile_pool(name="attn", bufs=1) as apl,
                tc.tile_pool(name="probs", bufs=8) as prp,
                tc.tile_pool(name="bcD", bufs=2) as bdp,
                tc.tile_pool(name="psS", bufs=4, space="PSUM") as psS,
                tc.tile_pool(name="psCtx", bufs=2, space="PSUM") as psC,
                tc.tile_pool(name="psSum", bufs=2, space="PSUM") as psU,
            ):
                ctx_sb = apl.tile([128, HPC * S], F32R, tag="ctx_sb")
                # single lower-triangle mask tile: tri[i, c] = 1 iff i <= c
                tri = apl.tile([128, 128], F32, tag="tri")
                nc.sync.dma_start(tri[:], masksT[0, :, 0:128])

                for m in range(HPC):
                    for s in range(NS):
                        n_tk = 4 * (s + 1)
                        ctx_ps = psC.tile([128, 512], F32, tag="ctx_ps", name="ctx_ps")
                        sum_ps = psU.tile([1, 512], F32, tag="sum_ps", name="sum_ps")
                        for j in range(n_tk):
                            p_rel = j - 4 * s
                            # diagonal blocks only attend to tq_local >= off
                            off = 128 * p_rel if p_rel >= 0 else 0
                            n = 512 - off
                            sc = psS.tile([128, 512], F32, tag="sc", name="sc")
                            nc.tensor.matmul(
                                sc[:, 0:n],
                                krope[:, ds(m * S + j * 128, 128)],
                                qrope[:, ds(m * S + s * 512 + off, n)],
                                start=True,
                                stop=True,
                            )
                            pr = prp.tile([128, 512], F32R, tag="probs", name="pr")
                            if p_rel >= 0:
                                # triangle (first 128 cols of the valid range)
                                et = prp.tile([128, 128], F32, tag="expt", name="et")
                                nc.scalar.activation(
                                    et[:], sc[:, 0:128], AF.Exp, scale=INV_SQRT_D
                                )
                                nc.vector.tensor_mul(pr[:, 0:128], et[:], tri[:])
                                if n > 128:
                                    nc.scalar.activation(
                                        pr[:, 128:n],
                                        sc[:, 128:n],
                                        AF.Exp,
                                        scale=INV_SQRT_D,
                                    )
                            else:
                                nc.scalar.activation(
                                    pr[:, 0:n], sc[:, 0:n], AF.Exp, scale=INV_SQRT_D
                                )
                            nc.tensor.matmul(
                                ctx_ps[:, ds(off, n)],
                                v_sb[:, ds(j * JC + m * D, D)],
                                pr[:, 0:n],
                                start=(j == 0),
                                stop=(j == n_tk - 1),
                            )
                            nc.tensor.matmul(
                                sum_ps[0:1, ds(off, n)],
                                ones_r[:],
                                pr[:, 0:n],
                                start=(j == 0),
                                stop=(j == n_tk - 1),
                            )
                        rr = bdp.tile([1, 512], F32, tag="recip", name="rr")
                        nc.vector.reciprocal(rr[:], sum_ps[:])
                        bc = bdp.tile([128, 512], F32, tag="bcD", name="bc")
                        nc.gpsimd.partition_broadcast(bc[:], rr[0:1, :])
                        nc.vector.tensor_mul(
                            ctx_sb[:, ds(m * S + s * 512, 512)], ctx_ps[:], bc[:]
                        )
                        nc.sync.dma_start(
                            cbounce[m][:, ts(s, 512)],
                            ctx_sb[:, ds(m * S + s * 512, 512)],
                        )
                    nc.gpsimd.collective_compute(
                        "AllGather",
                        ALU.bypass,
                        replica_groups=rg,
                        ins=[cbounce[m].opt()],
                        outs=[ag_out[m].opt()],
                    )

            # ---------------- Phase E: output projection ----------------
            with (
                tc.tile_pool(name="ck", bufs=ET) as ckp,
                tc.tile_pool(name="wo", bufs=1) as wop,
                tc.tile_pool(name="ob", bufs=2) as obp,
                tc.tile_pool(name="psW", bufs=3, space="PSUM") as psW,
            ):
                woT_sb = wop.tile([128, ET * EB], F32R, tag="woT_sb")
                nc.sync.dma_start(
                    woT_sb[:].rearrange("p (a j) -> p a j", a=ET),
                    woT[:, :].rearrange("(a p) j -> p a j", p=128),
                )
                ctxk = []
                for kb in range(ET):
                    ct = ckp.tile([128, S], F32R, tag="ck", name=f"ck{kb}")
                    src = ag_out[0] if kb < CH else ag_out[1]
                    nc.sync.dma_start(ct[:], src[ts(kb % CH, 128), :])
                    ctxk.append(ct)
                for t in range(TT):
                    ps = psW.tile([128, EB], F32, tag="wo_ps", name="wo_ps")
                    for kb in range(ET):
                        nc.tensor.matmul(
                            ps[:],
                            ctxk[kb][:, ts(t, 128)],
                            woT_sb[:, ts(kb, EB)],
                            start=(kb == 0),
                            stop=(kb == ET - 1),
                        )
                    ob = obp.tile([128, EB], F32, tag="ob", name="ob")
                    nc.vector.tensor_copy(ob[:], ps[:])
                    nc.sync.dma_start(out_ext[ts(t, 128), :], ob[:])

    nc.compile()
    return nc


def get_nc():
    if "nc" not in _NC_CACHE:
        _NC_CACHE["nc"] = _build_nc()
    return _NC_CACHE["nc"]


def _round_f32r(a):
    """Round fp32 to fp32r (11 explicit mantissa bits) with RNE."""
    u = np.ascontiguousarray(a, dtype=np.float32).view(np.uint32).copy()
    round_bit = (u >> 12) & 1
    u += 0x7FF + round_bit
    u &= np.uint32(0xFFFFF000)
    return u.view(np.float32)


def _rope_tables():
    """thetas with the reference's fp16-arange quirk, then f32 cos/sin."""
    try:
        # Same ops/dtypes as the reference, on the default jax device, so
        # the fp16 pow rounds identically to the reference run in this env.
        import jax.numpy as jnp

        th = (
            THETA ** (-jnp.arange(HALF, dtype=jnp.float16) / HALF)
        ).astype(jnp.float32)
        thetas = np.asarray(th)
    except Exception:
        ar = np.arange(HALF, dtype=np.float16)
        y = -ar / np.float16(HALF)
        thetas = (np.float16(THETA) ** y).astype(np.float32)
    m = np.arange(S, dtype=np.float32)
    ang = m[:, None] * thetas[None, :]  # [S, 64] f32
    cos = np.ascontiguousarray(np.cos(ang).astype(np.float32).T)  # [64, S]
    sin = np.ascontiguousarray(np.sin(ang).astype(np.float32).T)
    cosF = np.concatenate([cos, cos], axis=0)  # [128, S]
    sinF = np.concatenate([-sin, sin], axis=0)
    return np.ascontiguousarray(cosF), np.ascontiguousarray(sinF)


def _host_prep(xs, norm_w, wq, wk, wv, wo):
    xs = np.asarray(xs, dtype=np.float32)
    norm_w = np.asarray(norm_w, dtype=np.float32)
    wq = np.asarray(wq, dtype=np.float32)
    wk = np.asarray(wk, dtype=np.float32)
    wv = np.asarray(wv, dtype=np.float32)
    wo = np.asarray(wo, dtype=np.float32)

    xsT = _round_f32r(np.ascontiguousarray(xs.T))
    cosF, sinF = _rope_tables()

    i = np.arange(128)[:, None]
    tq = np.arange(512)[None, :]
    masks = np.stack(
        [((128 * p + i) <= tq).astype(np.float32) for p in range(NS)]
    )

    perm = np.concatenate([np.arange(0, D, 2), np.arange(1, D, 2)])
    wq_n = wq * norm_w[None, :]
    wk_n = wk * norm_w[None, :]
    wv_n = wv * norm_w[None, :]
    f_order = np.concatenate(
        [np.arange(h * D, (h + 1) * D) for h in range(0, H, 2)]
        + [np.arange(h * D, (h + 1) * D) for h in range(1, H, 2)]
    )

    in_maps = []
    for c in range(N_CORES):
        heads = (2 * c, 2 * c + 1)
        rows_qk = np.concatenate([h * D + perm for h in heads])
        rows_v = np.concatenate([np.arange(h * D, (h + 1) * D) for h in heads])
        in_maps.append(
            {
                "xsT": xsT,
                "wqT": _round_f32r(np.ascontiguousarray(wq_n[rows_qk].T)),
                "wkT": _round_f32r(np.ascontiguousarray(wk_n[rows_qk].T)),
                "wvT": _round_f32r(np.ascontiguousarray(wv_n[rows_v].T)),
                "woT": _round_f32r(
                    np.ascontiguousarray(wo[c * EB : (c + 1) * EB, :].T[f_order, :])
                ),
                "cosF": cosF,
                "sinF": sinF,
                "masks": masks,
            }
        )
    return in_maps


def kernel(xs, norm_w, wq, wk, wv, wo):
    from concourse.bass_utils import run_bass_kernel_spmd

    nc = get_nc()
    in_maps = _host_prep(xs, norm_w, wq, wk, wv, wo)
    res = run_bass_kernel_spmd(nc, in_maps, list(range(N_CORES)))
    out = np.concatenate([res.results[c]["out"] for c in range(N_CORES)], axis=1)
    return out.astype(np.float32)



# revision 2
# speedup vs baseline: 1.0756x; 1.0756x over previous
"""Trainium2 Bass kernel for nn_AttentionModule_53223234732422 (v2).

Computes: RMSNorm -> QKV projections -> interleaved-pair RoPE on Q,K ->
causal softmax attention (16 heads, head_dim 128) -> output projection.

Sharding (8 NeuronCores, tensor parallel over heads):
  - xs is shipped FEATURE-sharded (core c gets feature rows [256c, 256c+256)
    as bf16) and AllGathered on device into the full feature-major xsT,
  - each core owns 2 heads: QKV projections with column-sliced weights,
    RoPE, causal attention for those heads,
  - both heads' context go out in ONE AllGather ([128, 2S] bf16 per rank),
  - output projection is split column-wise: each core produces 256 output
    features from the full gathered context.

Host-side preparation (layout only):
  - ALL per-core inputs are packed into a single bf16 blob [128, 24576]:
    cols [0,4096) xs feature slice, [4096,20480) wq|wk|wv|wo packed
    per-e-tile in SBUF layout, [20480,24576) cos/sin rope tables,
  - norm_w folded into the QKV weights; wq/wk rows permuted per head so
    RoPE pairs are deinterleaved (scores are permutation invariant),
  - causal mask generated on device via affine_select (nothing shipped).

Everything computes in bf16 on the PE (PSUM accumulation fp32); the
1/rms per-token scale is folded into the rope tables (q,k) and the v
eviction. Output is bf16, widened to fp32 on the host.
"""

import sys

sys.path.insert(0, "/opt/trn_rl_repo")

import numpy as np

import concourse.bacc as bacc
import concourse.mybir as mybir
import concourse.tile as tile
from concourse.bass import ds, ts

dt = mybir.dt
AF = mybir.ActivationFunctionType
ALU = mybir.AluOpType

S = 2048
E = 2048
H = 16
D = 128
HALF = D // 2
EPS = 1e-6
THETA = 10000.0
N_CORES = 8
HPC = H // N_CORES  # 2 heads per core
JC = HPC * D  # 256: local q/k/v width
EB = E // N_CORES  # 256: output columns per core
ET = E // 128  # 16 feature tiles
TT = S // 128  # 16 token tiles
NS = S // 512  # 4 token strips
WBLK = 1024  # w_all per-e-tile block: wq 256 | wk 256 | wv 256 | wo 256
XS_COLS = HPC * S  # 4096
W0 = XS_COLS
TR0 = W0 + ET * WBLK  # 20480
BLOB_COLS = TR0 + 2 * S  # 24576
INV_SQRT_D = float(1.0 / np.sqrt(np.float32(D)))

F32 = dt.float32
BF16 = dt.bfloat16

_NC_CACHE = {}


def _build_nc():
    nc = bacc.Bacc(trn_type="TRN2", num_devices=N_CORES)

    blob = nc.dram_tensor("blob", [128, BLOB_COLS], BF16, kind="ExternalInput")
    out_ext = nc.dram_tensor("out", [S, EB], BF16, kind="ExternalOutput")

    rg = [list(range(N_CORES))]

    with tile.TileContext(nc) as tc:
        with (
            tc.tile_pool(name="persist", bufs=1) as pp,
            tc.tile_pool(name="dram", bufs=1, space="DRAM") as dpool,
        ):
            ones_f = pp.tile([128, 1], F32, tag="ones_f")
            ones_b = pp.tile([128, 1], BF16, tag="ones_b")
            eps_sc = pp.tile([1, 1], F32, tag="eps_sc")
            nc.vector.memset(ones_f[:], 1.0)
            nc.vector.tensor_copy(ones_b[:], ones_f[:])
            nc.vector.memset(eps_sc[:], EPS)

            qrope = pp.tile([128, HPC * S], BF16, tag="qrope")
            krope = pp.tile([128, HPC * S], BF16, tag="krope")
            v_sb = pp.tile([128, TT * JC], BF16, tag="v_sb")
            w_sb = pp.tile([128, ET * WBLK], BF16, tag="w_sb")

            xs_loc = dpool.tile([128, XS_COLS], BF16, tag="xs_loc", name="xs_loc")
            xs_ag = dpool.tile(
                [N_CORES * 128, XS_COLS],
                BF16,
                addr_space="Shared",
                tag="xs_ag",
                name="xs_ag",
            )
            cb = dpool.tile([128, HPC * S], BF16, tag="cb", name="cb")
            ag_ctx = dpool.tile(
                [N_CORES * 128, HPC * S],
                BF16,
                addr_space="Shared",
                tag="ag_ctx",
                name="ag_ctx",
            )

            # ------------- Phase 0: AllGather the xs feature slices -------
            with tc.tile_pool(name="x0", bufs=1) as x0p:
                xs_sb = x0p.tile([128, XS_COLS], BF16, tag="xs_sb")
                nc.sync.dma_start(xs_sb[:], blob[:, 0:XS_COLS])
                nc.sync.dma_start(xs_loc[:], xs_sb[:])
                nc.gpsimd.collective_compute(
                    "AllGather",
                    ALU.bypass,
                    replica_groups=rg,
                    ins=[xs_loc.opt()],
                    outs=[xs_ag.opt()],
                )
            nc.sync.dma_start(w_sb[:], blob[:, W0:TR0])

            # ------------- Phases A+C: rms + QKV ---------------------------
            with tc.tile_pool(name="bcC", bufs=1) as bcp:
                bcastR = bcp.tile([128, S], F32, tag="bcastR")
                recip_col = bcp.tile([128, TT], F32, tag="recip_col")
                cos_sb = bcp.tile([128, S], F32, tag="cos_sb")
                sin_sb = bcp.tile([128, S], F32, tag="sin_sb")
                cos_raw = bcp.tile([128, S], BF16, tag="cos_raw")
                sin_raw = bcp.tile([128, S], BF16, tag="sin_raw")
                nc.sync.dma_start(cos_raw[:], blob[:, TR0 : TR0 + S])
                nc.sync.dma_start(sin_raw[:], blob[:, TR0 + S : TR0 + 2 * S])
                nc.vector.tensor_copy(cos_sb[:], cos_raw[:])
                nc.vector.tensor_copy(sin_sb[:], sin_raw[:])

                with tc.tile_pool(name="xsp", bufs=ET) as xsp:
                    xt = []
                    for e in range(ET):
                        t_ = xsp.tile([128, S], BF16, tag="xt", name=f"xt{e}")
                        nc.sync.dma_start(
                            t_[:],
                            xs_ag[ds((e // HPC) * 128, 128), ds((e % HPC) * S, S)],
                        )
                        xt.append(t_)

                    # ssq -> rms -> 1/rms (folded into trig + v eviction)
                    with tc.tile_pool(name="rmsp", bufs=1) as rmsp:
                        rms_row = rmsp.tile([1, S], F32, tag="rms_row")
                        with (
                            tc.tile_pool(name="sqp", bufs=2) as sqp,
                            tc.tile_pool(name="psA", bufs=NS, space="PSUM") as psA,
                        ):
                            ssq_ps = [
                                psA.tile([1, 512], F32, tag="ssq", name="ssq")
                                for _ in range(NS)
                            ]
                            for e in range(ET):
                                sq = sqp.tile([128, S], BF16, tag="sq")
                                nc.vector.tensor_mul(sq[:], xt[e][:], xt[e][:])
                                for s in range(NS):
                                    nc.tensor.matmul(
                                        ssq_ps[s][:],
                                        ones_b[:],
                                        sq[:, ts(s, 512)],
                                        start=(e == 0),
                                        stop=(e == ET - 1),
                                    )
                            for s in range(NS):
                                nc.scalar.activation(
                                    rms_row[0:1, ts(s, 512)],
                                    ssq_ps[s][:],
                                    AF.Sqrt,
                                    bias=eps_sc[0:1, 0:1],
                                    scale=1.0 / E,
                                )
                        nc.vector.reciprocal(rms_row[:], rms_row[:])
                        nc.gpsimd.partition_broadcast(bcastR[:], rms_row[0:1, :])
                        rrow_d = dpool.tile([1, S], F32, tag="rrow_d", name="rrow_d")
                        nc.sync.dma_start(rrow_d[:], rms_row[:])
                        nc.sync.dma_start(
                            recip_col[:],
                            rrow_d[0, :].rearrange("(a p) -> p a", p=128),
                        )

                    # fold 1/rms into the rope tables
                    nc.vector.tensor_mul(cos_sb[:], cos_sb[:], bcastR[:])
                    nc.vector.tensor_mul(sin_sb[:], sin_sb[:], bcastR[:])

                    with (
                        tc.tile_pool(name="rsw", bufs=4) as rsp,
                        tc.tile_pool(name="psQK", bufs=4, space="PSUM") as psQK,
                        tc.tile_pool(name="psV", bufs=3, space="PSUM") as psV,
                    ):
                        # q and k projections -> d-major [d, t], rope at
                        # eviction (single contraction chunk; all 16 e-tiles
                        # are SBUF-resident in bf16)
                        for m in range(HPC):
                            for joff, rope_dst in ((0, qrope), (JC, krope)):
                                for s in range(NS):
                                    ps = psQK.tile(
                                        [128, 512], F32, tag="qk_ps", name="qk_ps"
                                    )
                                    for e in range(ET):
                                        nc.tensor.matmul(
                                            ps[:],
                                            w_sb[:, ds(e * WBLK + joff + m * D, D)],
                                            xt[e][:, ts(s, 512)],
                                            start=(e == 0),
                                            stop=(e == ET - 1),
                                        )
                                    # RoPE: r = cos*q + sin*swap64(q)
                                    sw = rsp.tile([128, 512], F32, tag="rsw", name="rsw")
                                    nc.vector.tensor_copy(sw[0:64, :], ps[64:128, :])
                                    nc.vector.tensor_copy(sw[64:128, :], ps[0:64, :])
                                    tmp = rsp.tile(
                                        [128, 512], F32, tag="rtmp", name="rtmp"
                                    )
                                    nc.vector.tensor_mul(
                                        tmp[:], ps[:], cos_sb[:, ts(s, 512)]
                                    )
                                    nc.vector.tensor_mul(
                                        sw[:], sw[:], sin_sb[:, ts(s, 512)]
                                    )
                                    nc.vector.tensor_tensor(
                                        rope_dst[:, ds(m * S + s * 512, 512)],
                                        tmp[:],
                                        sw[:],
                                        ALU.add,
                                    )

                        # v projection -> token-major [t, j], scaled by
                        # 1/rms[t] (per-partition scalar) at eviction
                        for t in range(TT):
                            ps = psV.tile([128, JC], F32, tag="v_ps", name="v_ps")
                            for e in range(ET):
                                nc.tensor.matmul(
                                    ps[:],
                                    xt[e][:, ts(t, 128)],
                                    w_sb[:, ds(e * WBLK + 2 * JC, JC)],
                                    start=(e == 0),
                                    stop=(e == ET - 1),
                                )
                            nc.vector.tensor_scalar_mul(
                                v_sb[:, ts(t, JC)], ps[:], recip_col[:, t : t + 1]
                            )

            # ---------------- Phase D: attention ----------------
            with (
                tc.tile_pool(name="attn", bufs=1) as apl,
                tc.tile_pool(name="probs", bufs=8) as prp,
                tc.tile_pool(name="bcD", bufs=2) as bdp,
                tc.tile_pool(name="psS", bufs=4, space="PSUM") as psS,
                tc.tile_pool(name="psCtx", bufs=2, space="PSUM") as psC,
                tc.tile_pool(name="psSum", bufs=2, space="PSUM") as psU,
            ):
                ctx_sb = apl.tile([128, HPC * S], BF16, tag="ctx_sb")

                for m in range(HPC):
                    for s in range(NS):
                        n_tk = 4 * (s + 1)
                        ctx_ps = psC.tile([128, 512], F32, tag="ctx_ps", name="ctx_ps")
                        sum_ps = psU.tile([1, 512], F32, tag="sum_ps", name="sum_ps")
                        for j in range(n_tk):
                            p_rel = j - 4 * s
                            # diagonal blocks only attend to tq_local >= off
                            off = 128 * p_rel if p_rel >= 0 else 0
                            n = 512 - off
                            sc = psS.tile([128, 512], F32, tag="sc", name="sc")
                            nc.tensor.matmul(
                                sc[:, 0:n],
                                krope[:, ds(m * S + j * 128, 128)],
                                qrope[:, ds(m * S + s * 512 + off, n)],
                                start=True,
                                stop=True,
                            )
                            pr = prp.tile([128, 512], BF16, tag="probs", name="pr")
                            if p_rel >= 0:
                                # triangle (first 128 cols of the valid range):
                                # keep where q_local >= k_local
                                et = prp.tile([128, 128], BF16, tag="expt", name="et")
                                nc.scalar.activation(
                                    et[:], sc[:, 0:128], AF.Exp, scale=INV_SQRT_D
                                )
                                nc.gpsimd.affine_select(
                                    pr[:, 0:128],
                                    et[:],
                                    pattern=[[1, 128]],
                                    compare_op=ALU.is_ge,
                                    fill=0.0,
                                    base=0,
                                    channel_multiplier=-1,
                                )
                                if n > 128:
                                    nc.scalar.activation(
                                        pr[:, 128:n],
                                        sc[:, 128:n],
                                        AF.Exp,
                                        scale=INV_SQRT_D,
                                    )
                            else:
                                nc.scalar.activation(
                                    pr[:, 0:n], sc[:, 0:n], AF.Exp, scale=INV_SQRT_D
                                )
                            nc.tensor.matmul(
                                ctx_ps[:, ds(off, n)],
                                v_sb[:, ds(j * JC + m * D, D)],
                                pr[:, 0:n],
                                start=(j == 0),
                                stop=(j == n_tk - 1),
                            )
                            nc.tensor.matmul(
                                sum_ps[0:1, ds(off, n)],
                                ones_b[:],
                                pr[:, 0:n],
                                start=(j == 0),
                                stop=(j == n_tk - 1),
                            )
                        rr = bdp.tile([1, 512], F32, tag="recip", name="rr")
                        nc.vector.reciprocal(rr[:], sum_ps[:])
                        bc = bdp.tile([128, 512], F32, tag="bcD", name="bc")
                        nc.gpsimd.partition_broadcast(bc[:], rr[0:1, :])
                        nc.vector.tensor_mul(
                            ctx_sb[:, ds(m * S + s * 512, 512)], ctx_ps[:], bc[:]
                        )
                nc.sync.dma_start(cb[:], ctx_sb[:])
                nc.gpsimd.collective_compute(
                    "AllGather",
                    ALU.bypass,
                    replica_groups=rg,
                    ins=[cb.opt()],
                    outs=[ag_ctx.opt()],
                )

            # ---------------- Phase E: output projection ----------------
            with (
                tc.tile_pool(name="ck", bufs=ET) as ckp,
                tc.tile_pool(name="ob", bufs=2) as obp,
                tc.tile_pool(name="psW", bufs=3, space="PSUM") as psW,
            ):
                ck = []
                for kb in range(ET):
                    ct = ckp.tile([128, S], BF16, tag="ck", name=f"ck{kb}")
                    nc.sync.dma_start(
                        ct[:],
                        ag_ctx[ds((kb // HPC) * 128, 128), ds((kb % HPC) * S, S)],
                    )
                    ck.append(ct)
                for t in range(TT):
                    ps = psW.tile([128, EB], F32, tag="wo_ps", name="wo_ps")
                    for kb in range(ET):
                        nc.tensor.matmul(
                            ps[:],
                            ck[kb][:, ts(t, 128)],
                            w_sb[:, ds(kb * WBLK + 3 * JC, EB)],
                            start=(kb == 0),
                            stop=(kb == ET - 1),
                        )
                    ob = obp.tile([128, EB], BF16, tag="ob", name="ob")
                    nc.vector.tensor_copy(ob[:], ps[:])
                    nc.sync.dma_start(out_ext[ts(t, 128), :], ob[:])

    nc.compile()
    return nc


def get_nc():
    if "nc" not in _NC_CACHE:
        _NC_CACHE["nc"] = _build_nc()
    return _NC_CACHE["nc"]


def _rope_tables():
    """thetas with the reference's fp16-arange quirk, then f32 cos/sin
    tables in deinterleaved row order: rows 0..63 carry the cos/-sin
    (x0) lanes, rows 64..127 the cos/+sin (x1) lanes."""
    try:
        import jax.numpy as jnp

        th = (
            THETA ** (-jnp.arange(HALF, dtype=jnp.float16) / HALF)
        ).astype(jnp.float32)
        thetas = np.asarray(th)
    except Exception:
        ar = np.arange(HALF, dtype=np.float16)
        y = -ar / np.float16(HALF)
        thetas = (np.float16(THETA) ** y).astype(np.float32)
    m = np.arange(S, dtype=np.float32)
    ang = m[:, None] * thetas[None, :]  # [S, 64] f32
    cos = np.ascontiguousarray(np.cos(ang).astype(np.float32).T)  # [64, S]
    sin = np.ascontiguousarray(np.sin(ang).astype(np.float32).T)
    cosF = np.concatenate([cos, cos], axis=0)  # [128, S]
    sinF = np.concatenate([-sin, sin], axis=0)
    return np.ascontiguousarray(cosF), np.ascontiguousarray(sinF)


def _host_prep(xs, norm_w, wq, wk, wv, wo):
    xs = np.asarray(xs, dtype=np.float32)
    norm_w = np.asarray(norm_w, dtype=np.float32)
    wq = np.asarray(wq, dtype=np.float32)
    wk = np.asarray(wk, dtype=np.float32)
    wv = np.asarray(wv, dtype=np.float32)
    wo = np.asarray(wo, dtype=np.float32)
    bf16 = mybir.dt.np(BF16)

    xsT = np.ascontiguousarray(xs.T)  # [E, S] feature-major
    cosF, sinF = _rope_tables()
    cos_b = cosF.astype(bf16)
    sin_b = sinF.astype(bf16)

    perm = np.concatenate([np.arange(0, D, 2), np.arange(1, D, 2)])
    wq_n = wq * norm_w[None, :]
    wk_n = wk * norm_w[None, :]
    wv_n = wv * norm_w[None, :]

    in_maps = []
    for c in range(N_CORES):
        heads = (HPC * c, HPC * c + 1)
        rows_qk = np.concatenate([h * D + perm for h in heads])
        rows_v = np.concatenate([np.arange(h * D, (h + 1) * D) for h in heads])

        blob = np.empty((128, BLOB_COLS), dtype=bf16)
        xsl = xsT[c * JC : (c + 1) * JC, :]  # [256, S] feature slice
        blob[:, 0:S] = xsl[0:128, :].astype(bf16)
        blob[:, S : 2 * S] = xsl[128:256, :].astype(bf16)
        wcat = np.concatenate(
            [
                wq_n[rows_qk].T,
                wk_n[rows_qk].T,
                wv_n[rows_v].T,
                wo[c * EB : (c + 1) * EB, :].T,
            ],
            axis=1,
        )  # [E, WBLK] f32; rows = contraction features, natural e-order
        blob[:, W0:TR0] = (
            wcat.reshape(ET, 128, WBLK)
            .transpose(1, 0, 2)
            .reshape(128, ET * WBLK)
            .astype(bf16)
        )
        blob[:, TR0 : TR0 + S] = cos_b
        blob[:, TR0 + S : TR0 + 2 * S] = sin_b
        in_maps.append({"blob": blob})
    return in_maps


def kernel(xs, norm_w, wq, wk, wv, wo):
    from concourse.bass_utils import run_bass_kernel_spmd

    nc = get_nc()
    in_maps = _host_prep(xs, norm_w, wq, wk, wv, wo)
    res = run_bass_kernel_spmd(nc, in_maps, list(range(N_CORES)))
    out = np.concatenate(
        [np.asarray(res.results[c]["out"]) for c in range(N_CORES)], axis=1
    )
    return out.astype(np.float32)


# revision 3
# speedup vs baseline: 1.1234x; 1.0444x over previous
"""Trainium2 Bass kernel for nn_AttentionModule_53223234732422 (v2).

Computes: RMSNorm -> QKV projections -> interleaved-pair RoPE on Q,K ->
causal softmax attention (16 heads, head_dim 128) -> output projection.

Sharding (8 NeuronCores, tensor parallel over heads):
  - xs is shipped FEATURE-sharded (core c gets feature rows [256c, 256c+256)
    as bf16) and AllGathered on device into the full feature-major xsT,
  - each core owns 2 heads: QKV projections with column-sliced weights,
    RoPE, causal attention for those heads,
  - both heads' context go out in ONE AllGather ([128, 2S] bf16 per rank),
  - output projection is split column-wise: each core produces 256 output
    features from the full gathered context.

Host-side preparation (layout only):
  - ALL per-core inputs are packed into a single bf16 blob [128, 24576]:
    cols [0,4096) xs feature slice, [4096,20480) wq|wk|wv|wo packed
    per-e-tile in SBUF layout, [20480,24576) cos/sin rope tables,
  - norm_w folded into the QKV weights; wq/wk rows permuted per head so
    RoPE pairs are deinterleaved (scores are permutation invariant),
  - causal mask generated on device via affine_select (nothing shipped).

Everything computes in bf16 on the PE (PSUM accumulation fp32); the
1/rms per-token scale is folded into the rope tables (q,k) and the v
eviction. Output is bf16, widened to fp32 on the host.
"""

import sys

sys.path.insert(0, "/opt/trn_rl_repo")

import numpy as np

import concourse.bacc as bacc
import concourse.mybir as mybir
import concourse.tile as tile
from concourse.bass import ds, ts

dt = mybir.dt
AF = mybir.ActivationFunctionType
ALU = mybir.AluOpType

S = 2048
E = 2048
H = 16
D = 128
HALF = D // 2
EPS = 1e-6
THETA = 10000.0
N_CORES = 8
HPC = H // N_CORES  # 2 heads per core
JC = HPC * D  # 256: local q/k/v width
EB = E // N_CORES  # 256: output columns per core
ET = E // 128  # 16 feature tiles
TT = S // 128  # 16 token tiles
NS = S // 512  # 4 token strips
WBLK = 1024  # w_all per-e-tile block: wq 256 | wk 256 | wv 256 | wo 256
XS_COLS = HPC * S  # 4096
W0 = XS_COLS
TR0 = W0 + ET * WBLK  # 20480
NA = S // 128  # 16 coarse angle steps
TRIG_F32 = 2 * NA + 2 * 128  # 288 f32 cols: Ac | As | Bc | Bs
BLOB_COLS = TR0 + 2 * TRIG_F32  # 21056 (trig shipped as f32 via bitcast)
INV_SQRT_D = float(1.0 / np.sqrt(np.float32(D)))

F32 = dt.float32
BF16 = dt.bfloat16

_NC_CACHE = {}


def _build_nc():
    nc = bacc.Bacc(trn_type="TRN2", num_devices=N_CORES)

    blob = nc.dram_tensor("blob", [128, BLOB_COLS], BF16, kind="ExternalInput")
    out_ext = nc.dram_tensor("out", [S, EB], BF16, kind="ExternalOutput")

    rg = [list(range(N_CORES))]

    with tile.TileContext(nc) as tc:
        with (
            tc.tile_pool(name="persist", bufs=1) as pp,
            tc.tile_pool(name="dram", bufs=1, space="DRAM") as dpool,
        ):
            ones_f = pp.tile([128, 1], F32, tag="ones_f")
            ones_b = pp.tile([128, 1], BF16, tag="ones_b")
            eps_sc = pp.tile([1, 1], F32, tag="eps_sc")
            nc.vector.memset(ones_f[:], 1.0)
            nc.vector.tensor_copy(ones_b[:], ones_f[:])
            nc.vector.memset(eps_sc[:], EPS)

            qrope = pp.tile([128, HPC * S], BF16, tag="qrope")
            krope = pp.tile([128, HPC * S], BF16, tag="krope")
            v_sb = pp.tile([128, TT * JC], BF16, tag="v_sb")
            w_sb = pp.tile([128, ET * WBLK], BF16, tag="w_sb")

            xs_loc = dpool.tile([128, XS_COLS], BF16, tag="xs_loc", name="xs_loc")
            xs_ag = dpool.tile(
                [N_CORES * 128, XS_COLS],
                BF16,
                addr_space="Shared",
                tag="xs_ag",
                name="xs_ag",
            )
            cb = dpool.tile([128, HPC * S], BF16, tag="cb", name="cb")
            ag_ctx = dpool.tile(
                [N_CORES * 128, HPC * S],
                BF16,
                addr_space="Shared",
                tag="ag_ctx",
                name="ag_ctx",
            )

            # ------------- Phase 0: AllGather the xs feature slices -------
            with tc.tile_pool(name="x0", bufs=1) as x0p:
                xs_sb = x0p.tile([128, XS_COLS], BF16, tag="xs_sb")
                nc.sync.dma_start(xs_sb[:], blob[:, 0:XS_COLS])
                nc.sync.dma_start(xs_loc[:], xs_sb[:])
                nc.gpsimd.collective_compute(
                    "AllGather",
                    ALU.bypass,
                    replica_groups=rg,
                    ins=[xs_loc.opt()],
                    outs=[xs_ag.opt()],
                )
            nc.sync.dma_start(w_sb[:], blob[:, W0:TR0])

            # ------------- Phases A+C: rms + QKV ---------------------------
            with tc.tile_pool(name="bcC", bufs=1) as bcp:
                bcastR = bcp.tile([128, S], F32, tag="bcastR")
                recip_col = bcp.tile([128, TT], F32, tag="recip_col")
                cos_sb = bcp.tile([128, S], F32, tag="cos_sb")
                sin_sb = bcp.tile([128, S], F32, tag="sin_sb")
                # rope tables from the f32 angle-addition decomposition:
                # cos(t*th) = Ac[a]Bc[b] - As[a]Bs[b], t = 128a + b (exact)
                trig = bcp.tile([128, TRIG_F32], F32, tag="trig")
                nc.sync.dma_start(
                    trig[:], blob[:, TR0 : TR0 + 2 * TRIG_F32].bitcast(F32)
                )
                with tc.tile_pool(name="trtmp", bufs=4) as trp:
                    Bc = trig[:, 2 * NA : 2 * NA + 128]
                    Bs = trig[:, 2 * NA + 128 : 2 * NA + 256]
                    for a in range(NA):
                        sl = ds(a * 128, 128)
                        ac = trig[:, a : a + 1]
                        as_ = trig[:, NA + a : NA + a + 1]
                        t1 = trp.tile([128, 128], F32, tag="t1", name="t1")
                        nc.vector.tensor_scalar_mul(t1[:], Bs, as_)
                        nc.vector.scalar_tensor_tensor(
                            cos_sb[:, sl], Bc, ac, t1[:], ALU.mult, ALU.subtract
                        )
                        t2 = trp.tile([128, 128], F32, tag="t2", name="t2")
                        nc.vector.tensor_scalar_mul(t2[:], Bc, as_)
                        nc.vector.scalar_tensor_tensor(
                            sin_sb[:, sl], Bs, ac, t2[:], ALU.mult, ALU.add
                        )

                with tc.tile_pool(name="xsp", bufs=ET) as xsp:
                    xt = []
                    for e in range(ET):
                        t_ = xsp.tile([128, S], BF16, tag="xt", name=f"xt{e}")
                        nc.sync.dma_start(
                            t_[:],
                            xs_ag[ds((e // HPC) * 128, 128), ds((e % HPC) * S, S)],
                        )
                        xt.append(t_)

                    # ssq -> rms -> 1/rms (folded into trig + v eviction)
                    with tc.tile_pool(name="rmsp", bufs=1) as rmsp:
                        rms_row = rmsp.tile([1, S], F32, tag="rms_row")
                        with (
                            tc.tile_pool(name="sqp", bufs=2) as sqp,
                            tc.tile_pool(name="psA", bufs=NS, space="PSUM") as psA,
                        ):
                            ssq_ps = [
                                psA.tile([1, 512], F32, tag="ssq", name="ssq")
                                for _ in range(NS)
                            ]
                            for e in range(ET):
                                sq = sqp.tile([128, S], BF16, tag="sq")
                                nc.vector.tensor_mul(sq[:], xt[e][:], xt[e][:])
                                for s in range(NS):
                                    nc.tensor.matmul(
                                        ssq_ps[s][:],
                                        ones_b[:],
                                        sq[:, ts(s, 512)],
                                        start=(e == 0),
                                        stop=(e == ET - 1),
                                    )
                            for s in range(NS):
                                nc.scalar.activation(
                                    rms_row[0:1, ts(s, 512)],
                                    ssq_ps[s][:],
                                    AF.Sqrt,
                                    bias=eps_sc[0:1, 0:1],
                                    scale=1.0 / E,
                                )
                        nc.vector.reciprocal(rms_row[:], rms_row[:])
                        nc.gpsimd.partition_broadcast(bcastR[:], rms_row[0:1, :])
                        rrow_d = dpool.tile([1, S], F32, tag="rrow_d", name="rrow_d")
                        nc.sync.dma_start(rrow_d[:], rms_row[:])
                        nc.sync.dma_start(
                            recip_col[:],
                            rrow_d[0, :].rearrange("(a p) -> p a", p=128),
                        )

                    # fold 1/rms into the rope tables
                    nc.vector.tensor_mul(cos_sb[:], cos_sb[:], bcastR[:])
                    nc.vector.tensor_mul(sin_sb[:], sin_sb[:], bcastR[:])

                    with (
                        tc.tile_pool(name="rsw", bufs=4) as rsp,
                        tc.tile_pool(name="psQK", bufs=4, space="PSUM") as psQK,
                        tc.tile_pool(name="psV", bufs=3, space="PSUM") as psV,
                    ):
                        # q and k projections -> d-major [d, t], rope at
                        # eviction (single contraction chunk; all 16 e-tiles
                        # are SBUF-resident in bf16)
                        for m in range(HPC):
                            for joff, rope_dst in ((0, qrope), (JC, krope)):
                                for s in range(NS):
                                    ps = psQK.tile(
                                        [128, 512], F32, tag="qk_ps", name="qk_ps"
                                    )
                                    for e in range(ET):
                                        nc.tensor.matmul(
                                            ps[:],
                                            w_sb[:, ds(e * WBLK + joff + m * D, D)],
                                            xt[e][:, ts(s, 512)],
                                            start=(e == 0),
                                            stop=(e == ET - 1),
                                        )
                                    # RoPE: r = cos*q + sin*swap64(q)
                                    sw = rsp.tile([128, 512], F32, tag="rsw", name="rsw")
                                    nc.vector.tensor_copy(sw[0:64, :], ps[64:128, :])
                                    nc.vector.tensor_copy(sw[64:128, :], ps[0:64, :])
                                    tmp = rsp.tile(
                                        [128, 512], F32, tag="rtmp", name="rtmp"
                                    )
                                    nc.vector.tensor_mul(
                                        tmp[:], ps[:], cos_sb[:, ts(s, 512)]
                                    )
                                    nc.vector.tensor_mul(
                                        sw[:], sw[:], sin_sb[:, ts(s, 512)]
                                    )
                                    nc.vector.tensor_tensor(
                                        rope_dst[:, ds(m * S + s * 512, 512)],
                                        tmp[:],
                                        sw[:],
                                        ALU.add,
                                    )

                        # v projection -> token-major [t, j], scaled by
                        # 1/rms[t] (per-partition scalar) at eviction
                        for t in range(TT):
                            ps = psV.tile([128, JC], F32, tag="v_ps", name="v_ps")
                            for e in range(ET):
                                nc.tensor.matmul(
                                    ps[:],
                                    xt[e][:, ts(t, 128)],
                                    w_sb[:, ds(e * WBLK + 2 * JC, JC)],
                                    start=(e == 0),
                                    stop=(e == ET - 1),
                                )
                            nc.vector.tensor_scalar_mul(
                                v_sb[:, ts(t, JC)], ps[:], recip_col[:, t : t + 1]
                            )

            # ---------------- Phase D: attention ----------------
            with (
                tc.tile_pool(name="attn", bufs=1) as apl,
                tc.tile_pool(name="probs", bufs=8) as prp,
                tc.tile_pool(name="bcD", bufs=2) as bdp,
                tc.tile_pool(name="psS", bufs=4, space="PSUM") as psS,
                tc.tile_pool(name="psCtx", bufs=2, space="PSUM") as psC,
                tc.tile_pool(name="psSum", bufs=2, space="PSUM") as psU,
            ):
                ctx_sb = apl.tile([128, HPC * S], BF16, tag="ctx_sb")

                for m in range(HPC):
                    for s in range(NS):
                        n_tk = 4 * (s + 1)
                        ctx_ps = psC.tile([128, 512], F32, tag="ctx_ps", name="ctx_ps")
                        sum_ps = psU.tile([1, 512], F32, tag="sum_ps", name="sum_ps")
                        for j in range(n_tk):
                            p_rel = j - 4 * s
                            # diagonal blocks only attend to tq_local >= off
                            off = 128 * p_rel if p_rel >= 0 else 0
                            n = 512 - off
                            sc = psS.tile([128, 512], F32, tag="sc", name="sc")
                            nc.tensor.matmul(
                                sc[:, 0:n],
                                krope[:, ds(m * S + j * 128, 128)],
                                qrope[:, ds(m * S + s * 512 + off, n)],
                                start=True,
                                stop=True,
                            )
                            pr = prp.tile([128, 512], BF16, tag="probs", name="pr")
                            if p_rel >= 0:
                                # triangle (first 128 cols of the valid range):
                                # keep where q_local >= k_local
                                et = prp.tile([128, 128], BF16, tag="expt", name="et")
                                nc.scalar.activation(
                                    et[:], sc[:, 0:128], AF.Exp, scale=INV_SQRT_D
                                )
                                nc.gpsimd.affine_select(
                                    pr[:, 0:128],
                                    et[:],
                                    pattern=[[1, 128]],
                                    compare_op=ALU.is_ge,
                                    fill=0.0,
                                    base=0,
                                    channel_multiplier=-1,
                                )
                                if n > 128:
                                    nc.scalar.activation(
                                        pr[:, 128:n],
                                        sc[:, 128:n],
                                        AF.Exp,
                                        scale=INV_SQRT_D,
                                    )
                            else:
                                nc.scalar.activation(
                                    pr[:, 0:n], sc[:, 0:n], AF.Exp, scale=INV_SQRT_D
                                )
                            nc.tensor.matmul(
                                ctx_ps[:, ds(off, n)],
                                v_sb[:, ds(j * JC + m * D, D)],
                                pr[:, 0:n],
                                start=(j == 0),
                                stop=(j == n_tk - 1),
                            )
                            nc.tensor.matmul(
                                sum_ps[0:1, ds(off, n)],
                                ones_b[:],
                                pr[:, 0:n],
                                start=(j == 0),
                                stop=(j == n_tk - 1),
                            )
                        rr = bdp.tile([1, 512], F32, tag="recip", name="rr")
                        nc.vector.reciprocal(rr[:], sum_ps[:])
                        bc = bdp.tile([128, 512], F32, tag="bcD", name="bc")
                        nc.gpsimd.partition_broadcast(bc[:], rr[0:1, :])
                        nc.vector.tensor_mul(
                            ctx_sb[:, ds(m * S + s * 512, 512)], ctx_ps[:], bc[:]
                        )
                nc.sync.dma_start(cb[:], ctx_sb[:])
                nc.gpsimd.collective_compute(
                    "AllGather",
                    ALU.bypass,
                    replica_groups=rg,
                    ins=[cb.opt()],
                    outs=[ag_ctx.opt()],
                )

            # ---------------- Phase E: output projection ----------------
            with (
                tc.tile_pool(name="ck", bufs=ET) as ckp,
                tc.tile_pool(name="ob", bufs=2) as obp,
                tc.tile_pool(name="psW", bufs=3, space="PSUM") as psW,
            ):
                ck = []
                for kb in range(ET):
                    ct = ckp.tile([128, S], BF16, tag="ck", name=f"ck{kb}")
                    nc.sync.dma_start(
                        ct[:],
                        ag_ctx[ds((kb // HPC) * 128, 128), ds((kb % HPC) * S, S)],
                    )
                    ck.append(ct)
                for t in range(TT):
                    ps = psW.tile([128, EB], F32, tag="wo_ps", name="wo_ps")
                    for kb in range(ET):
                        nc.tensor.matmul(
                            ps[:],
                            ck[kb][:, ts(t, 128)],
                            w_sb[:, ds(kb * WBLK + 3 * JC, EB)],
                            start=(kb == 0),
                            stop=(kb == ET - 1),
                        )
                    ob = obp.tile([128, EB], BF16, tag="ob", name="ob")
                    nc.vector.tensor_copy(ob[:], ps[:])
                    nc.sync.dma_start(out_ext[ts(t, 128), :], ob[:])

    nc.compile()
    return nc


def get_nc():
    if "nc" not in _NC_CACHE:
        _NC_CACHE["nc"] = _build_nc()
    return _NC_CACHE["nc"]


def _trig_tables():
    """thetas with the reference's fp16-arange quirk, decomposed into
    coarse/fine f32 cos/sin factor tables [128, 288]: Ac | As | Bc | Bs,
    where t = 128a + b and the device reconstructs
      cosF[d,t] = Ac[d,a]Bc[d,b] - As[d,a]Bs[d,b]
      sinF[d,t] = As[d,a]Bc[d,b] + Ac[d,a]Bs[d,b]
    The -sin/+sin deinterleaved-lane signs are folded into As/Bs."""
    try:
        import jax.numpy as jnp

        th = (
            THETA ** (-jnp.arange(HALF, dtype=jnp.float16) / HALF)
        ).astype(jnp.float32)
        thetas = np.asarray(th)
    except Exception:
        ar = np.arange(HALF, dtype=np.float16)
        y = -ar / np.float16(HALF)
        thetas = (np.float16(THETA) ** y).astype(np.float32)
    th_full = np.concatenate([thetas, thetas]).astype(np.float64)  # [128]
    s_d = np.concatenate([-np.ones(HALF), np.ones(HALF)])  # [128]
    a_idx = np.arange(NA, dtype=np.float64) * 128.0
    b_idx = np.arange(128, dtype=np.float64)
    ang_a = th_full[:, None] * a_idx[None, :]  # [128, NA]
    ang_b = th_full[:, None] * b_idx[None, :]  # [128, 128]
    Ac = np.cos(ang_a)
    As = s_d[:, None] * np.sin(ang_a)
    Bc = np.cos(ang_b)
    Bs = s_d[:, None] * np.sin(ang_b)
    trig = np.concatenate([Ac, As, Bc, Bs], axis=1).astype(np.float32)
    return np.ascontiguousarray(trig)  # [128, TRIG_F32]


def _host_prep(xs, norm_w, wq, wk, wv, wo):
    xs = np.asarray(xs, dtype=np.float32)
    norm_w = np.asarray(norm_w, dtype=np.float32)
    wq = np.asarray(wq, dtype=np.float32)
    wk = np.asarray(wk, dtype=np.float32)
    wv = np.asarray(wv, dtype=np.float32)
    wo = np.asarray(wo, dtype=np.float32)
    bf16 = mybir.dt.np(BF16)

    xsT = np.ascontiguousarray(xs.T)  # [E, S] feature-major
    trig_b = _trig_tables().view(bf16)  # [128, 2*TRIG_F32] raw-byte view

    perm = np.concatenate([np.arange(0, D, 2), np.arange(1, D, 2)])
    wq_n = wq * norm_w[None, :]
    wk_n = wk * norm_w[None, :]
    wv_n = wv * norm_w[None, :]

    in_maps = []
    for c in range(N_CORES):
        heads = (HPC * c, HPC * c + 1)
        rows_qk = np.concatenate([h * D + perm for h in heads])
        rows_v = np.concatenate([np.arange(h * D, (h + 1) * D) for h in heads])

        blob = np.empty((128, BLOB_COLS), dtype=bf16)
        xsl = xsT[c * JC : (c + 1) * JC, :]  # [256, S] feature slice
        blob[:, 0:S] = xsl[0:128, :].astype(bf16)
        blob[:, S : 2 * S] = xsl[128:256, :].astype(bf16)
        wcat = np.concatenate(
            [
                wq_n[rows_qk].T,
                wk_n[rows_qk].T,
                wv_n[rows_v].T,
                wo[c * EB : (c + 1) * EB, :].T,
            ],
            axis=1,
        )  # [E, WBLK] f32; rows = contraction features, natural e-order
        blob[:, W0:TR0] = (
            wcat.reshape(ET, 128, WBLK)
            .transpose(1, 0, 2)
            .reshape(128, ET * WBLK)
            .astype(bf16)
        )
        blob[:, TR0 : TR0 + 2 * TRIG_F32] = trig_b
        in_maps.append({"blob": blob})
    return in_maps


def kernel(xs, norm_w, wq, wk, wv, wo):
    from concourse.bass_utils import run_bass_kernel_spmd

    nc = get_nc()
    in_maps = _host_prep(xs, norm_w, wq, wk, wv, wo)
    res = run_bass_kernel_spmd(nc, in_maps, list(range(N_CORES)))
    out = np.concatenate(
        [np.asarray(res.results[c]["out"]) for c in range(N_CORES)], axis=1
    )
    return out.astype(np.float32)
